# revision 29
# baseline (speedup 1.0000x reference)
"""Trainium2 Bass kernel for nn_AttentionLayer: self-attention with Q=K=V.

Reference math (per batch element n, head h, d=64, L=1024):
    q_h   = x[:, 64h:64h+64]                      # (L, 64)
    S_h   = q_h @ q_h.T                           # (L, L), symmetric
    A_h   = softmax(S_h / 8, axis=-1)
    out_h = A_h @ q_h                             # (L, 64)
    out   = concat_h out_h                        # (L, 1024)
    attn  = mean_h A_h                            # (L, L)

Device strategy (one batch element per NeuronCore, 8 cores):
  - xT built on-device via PE transposes (DMA transpose unsupported for fp32).
  - S_h per 128-row block via fp32r matmuls (full-rate at N=512).
  - exp via ACT with accum_out -> row sums r (softmax denominators) for free.
    No max-subtraction: scores/8 are bounded (~|12|), exp is safe in fp32.
  - E_h (unnormalized exp) is symmetric, so the same SBUF buffer serves as
    both E[l, s] and E[s, l]; the AV matmul needs no transpose.
  - attn accumulated on DVE: M += E_h * (1/(H r_h[l])) via scalar_tensor_tensor
    (per-partition scalar), fully normalized on device. In fast mode E and the
    accumulator are bf16 (2x DVE) and the last head writes fp32.
  - out computed transposed (outT = q_h.T @ E_h, PE with q stationary);
    the per-row softmax division by r and the final transpose happen on host
    at gather time (pure rescale + layout, ~0.02% of FLOPs).
"""

import numpy as np

N_BATCH, L_SEQ, D_MODEL, N_HEADS = 8, 1024, 1024, 16
D_HEAD = D_MODEL // N_HEADS  # 64
N_CORES = 8
# "fast":    bf16 E + bf16 attn accumulator (TS+TT decomposition, 2-4x DVE)
# "hybrid":  bf16 E (fast matmuls) + fp32 attn accumulator via STT (1x DVE)
# "precise": fp32r E + fp32 accumulator
MODE = "precise"

_compiled = None


def _build(L=L_SEQ, H=N_HEADS, reps=1, mode=MODE):
    fast = mode == "fast"
    bf_e = mode in ("fast", "hybrid")
    import concourse.bacc as bacc
    import concourse.tile as tile
    import concourse.mybir as mybir
    from concourse.masks import make_identity

    fp32 = mybir.dt.float32
    fp32r = mybir.dt.float32r
    bf16 = mybir.dt.bfloat16
    e_dt = bf16 if bf_e else fp32r
    Exp = mybir.ActivationFunctionType.Exp
    mult = mybir.AluOpType.mult
    add = mybir.AluOpType.add

    P = 128
    D = D_HEAD
    G = H // 2              # head pairs (two heads share a 128-row xT block)
    DM = H * D              # model dim on this core
    B = L // P              # 128-row blocks of L
    NT = (L + 511) // 512   # moving-operand tiles per L
    NS = min(512, L)        # moving tile width

    nc = bacc.Bacc("TRN2")
    x_d = nc.declare_dram_parameter("x", [L, DM], fp32r, isOutput=False)
    outT_d = nc.declare_dram_parameter("outT", [DM, L], fp32, isOutput=True)
    attn_d = nc.declare_dram_parameter("attn", [L, L], fp32, isOutput=True)
    r_d = nc.declare_dram_parameter("r", [P, H * B], fp32, isOutput=True)

    with tile.TileContext(nc) as tc:
      for _rep in range(reps):
        with tc.tile_pool(name="singles", bufs=1) as singles:
            ident = singles.tile([P, P], fp32)
            make_identity(nc, ident)
            x_sb = singles.tile([P, B, DM], fp32r)    # x[b*128+p, c]
            xt_sb = singles.tile([P, G, L], fp32r)    # x[l, g*128+p]
            macc_f = singles.tile([P, B, L], fp32)    # attn[b*128+p, s] (final)
            if bf_e:
                x_bf = singles.tile([P, B, DM], bf16, tag="x_bf")
            else:
                x_bf = x_sb
            if fast:
                macc = singles.tile([P, B, L], bf16, tag="macc_bf")
            else:
                macc = macc_f
            r_all = singles.tile([P, H * B], fp32)    # r_h[b*128+p] at col h*B+b
            c_all = singles.tile([P, H * B], fp32)    # 1/(H r)

            x_view = x_d.rearrange("(b p) c -> p b c", p=P)
            for b in range(B):
                nc.sync.dma_start(out=x_sb[:, b, :], in_=x_view[:, b, :])
            if bf_e:
                for b in range(B):
                    nc.gpsimd.tensor_copy(
                        out=x_bf[:, b, :], in_=x_sb[:, b, :].bitcast(fp32)
                    )

            with (
                tc.tile_pool(name="e_pool", bufs=2) as e_pool,
                tc.tile_pool(name="o_stage", bufs=2) as o_stage,
                tc.tile_pool(name="s_psum", bufs=2, space="PSUM") as s_psum,
                tc.tile_pool(name="av_psum", bufs=2, space="PSUM") as av_psum,
            ):
                # Build xT with PE transposes (psum slots shared with S tiles);
                # evacuate on ACT (its startup slack) with a few on DVE.
                for g in range(G):
                    for i in range(B):
                        j = g * B + i
                        if j % 2 == 0:
                            ps = s_psum.tile([P, L], fp32, tag="S")
                        else:
                            ps = av_psum.tile([P, L], fp32, tag="O")
                        nc.tensor.transpose(
                            ps[:, :P], x_sb[:, i, g * P:(g + 1) * P].bitcast(fp32),
                            ident,
                        )
                        dst = xt_sb[:, g, i * P:(i + 1) * P]
                        if j % 4 != 3:
                            nc.vector.tensor_copy(out=dst, in_=ps[:, :P])
                        else:
                            nc.scalar.copy(out=dst, in_=ps[:, :P])

                def qkt_exp(h, E):
                    g, half = h // 2, h % 2
                    po = half * D
                    for b in range(B):
                        s_ps = s_psum.tile([P, L], fp32, tag="S")
                        for t in range(NT):
                            nc.tensor.matmul(
                                s_ps[:, t * NS:(t + 1) * NS],
                                lhsT=xt_sb[po:po + D, g, b * P:(b + 1) * P],
                                rhs=xt_sb[po:po + D, g, t * NS:(t + 1) * NS],
                                start=True, stop=True,
                            )
                        nc.scalar.activation(
                            out=E[:, b, :], in_=s_ps, func=Exp, scale=0.125,
                            accum_out=r_all[:, h * B + b:h * B + b + 1],
                        )

                def accum_av(h, E, scaled_pool):
                    # c = 1/(H r)
                    rcol = r_all[:, h * B:(h + 1) * B]
                    ccol = c_all[:, h * B:(h + 1) * B]
                    nc.vector.reciprocal(out=ccol, in_=rcol)
                    nc.vector.tensor_scalar_mul(ccol, ccol, 1.0 / H)

                    # attn accumulation: macc += E * c  (per-partition scalar).
                    # scalar_tensor_tensor has no fast DVE modes; in fast mode
                    # decompose into tensor_scalar (4x bf16) + tensor_tensor
                    # (2x bf16) instead.
                    last = h == H - 1
                    for b in range(B):
                        cs = c_all[:, h * B + b:h * B + b + 1]
                        Eb = E[:, b, :] if bf_e else E[:, b, :].bitcast(fp32)
                        dst = macc_f if (last or not fast) else macc
                        if h == 0:
                            nc.vector.tensor_scalar_mul(dst[:, b, :], Eb, cs)
                        elif fast:
                            tmp = scaled_pool.tile([P, L], bf16, tag="tmp")
                            nc.vector.tensor_scalar_mul(tmp[:], Eb, cs)
                            nc.vector.tensor_tensor(
                                out=dst[:, b, :], in0=macc[:, b, :], in1=tmp[:],
                                op=add,
                            )
                        else:
                            nc.vector.scalar_tensor_tensor(
                                out=dst[:, b, :], in0=Eb, scalar=cs,
                                in1=macc[:, b, :], op0=mult, op1=add,
                            )

                    # outT_h = q_h.T @ E_h   (E symmetric: buffer serves as E[s, l])
                    o_ps = av_psum.tile([D, L], fp32, tag="O")
                    for k in range(B):
                        for t in range(NT):
                            nc.tensor.matmul(
                                o_ps[:, t * NS:(t + 1) * NS],
                                lhsT=x_bf[:, k, h * D:(h + 1) * D],
                                rhs=E[:, k, t * NS:(t + 1) * NS],
                                start=(k == 0), stop=(k == B - 1),
                            )
                    o_sb = o_stage.tile([D, L], fp32, tag="o_sb")
                    nc.vector.tensor_copy(out=o_sb[:], in_=o_ps[:])
                    nc.sync.dma_start(out=outT_d[h * D:(h + 1) * D, :], in_=o_sb[:])

                attn_view = attn_d.rearrange("(b p) s -> p b s", p=P)
                for h in range(H):
                    E = e_pool.tile([P, B, L], e_dt, tag="E")
                    qkt_exp(h, E)
                    accum_av(h, E, o_stage)
                    if h == H - 1 and H > 1:
                        for b in range(B):
                            nc.sync.dma_start(
                                out=attn_view[:, b, :], in_=macc_f[:, b, :]
                            )
                if H == 1:
                    nc.sync.dma_start(out=attn_view[:], in_=macc_f[:])
                nc.sync.dma_start(out=r_d[:, :], in_=r_all[:])

    nc.compile()
    return nc


def _get_compiled():
    global _compiled
    if _compiled is None:
        _compiled = _build()
    return _compiled


def kernel(input_data):
    from concourse.bass_utils import run_bass_kernel_spmd

    x = np.asarray(input_data, dtype=np.float32)
    assert x.shape == (N_BATCH, L_SEQ, D_MODEL)
    nc = _get_compiled()

    in_maps = [{"x": x[i]} for i in range(N_CORES)]
    res = run_bass_kernel_spmd(nc, in_maps, list(range(N_CORES)))

    H, D, B, P = N_HEADS, D_HEAD, L_SEQ // 128, 128
    outs = np.empty((N_BATCH, L_SEQ, D_MODEL), np.float32)
    attns = np.empty((N_BATCH, L_SEQ, L_SEQ), np.float32)
    for i in range(N_CORES):
        outT = res.results[i]["outT"]          # (D_MODEL, L) = out.T, pre-softmax-div
        attn = res.results[i]["attn"]          # (L, L), fully normalized
        r = res.results[i]["r"]                # (128, H*B): r_h[b*128+p] at [p, h*B+b]
        r_hl = np.transpose(r.reshape(P, H, B), (1, 2, 0)).reshape(H, L_SEQ)
        out = (outT.reshape(H, D, L_SEQ) / r_hl[:, None, :]).reshape(D_MODEL, L_SEQ).T
        outs[i] = out
        attns[i] = attn
    return outs, attns
